# revision 27
# baseline (speedup 1.0000x reference)
"""GIN graph encoder (DispatchGraphEncoder) on 8 Trainium2 NeuronCores.

Strategy (node-sharded, SPMD, all fp32):
- Nodes split contiguously across 8 cores (12500 each, padded to 12544).
- Per layer: ncfw AllGather replicates h into a full table T on every core.
- Aggregation: each core's incident edges are sorted by (src-segment, dst),
  padded into 128-edge tiles that are src-segment-pure and dst-window-pure.
  dma_gather (int16 idx relative to a <=32767-row table segment) pulls the
  1KB source-feature rows; a host-built one-hot selection matrix S
  (shipped uint8, cast to f32 during DMA) right-multiplies each tile on the
  PE, accumulating per-dst-window sums in PSUM; windows accumulate across
  segments in an SBUF agg buffer.
- z = agg + (1+eps)h is built transposed (feature-major) in PSUM via
  PE transpose-matmuls; the 2-layer GIN MLP runs feature-major (weights as
  lhsT), relu+bias fused on the ACT engine; output is transposed back and
  written node-major for the next AllGather.
- Pooling: matmul against a host-built graph-membership matrix, tiny
  AllReduce, replicated final head; core 0's [64, 512] output is returned.

The SPMD program structure (tile counts per (segment, window) run) is made
uniform across cores by padding to the max core's count; dead slots carry
idx=0 and an all-zero S column.
"""
import os
import sys

import numpy as np
import ml_dtypes

sys.path.insert(0, "/opt/trn_rl_repo")

from concourse import bass, bacc, mybir, tile  # noqa: E402
from concourse.masks import make_identity  # noqa: E402

P = 128


def full_cfg():
    return dict(
        N=100000, E=800000, D=128, H=256, OUT=512, L=4, G=64, NCORES=8,
        RN=12500, BLK=14336, NSEG=4, CALL_TILES=8,
    )


def tiny_cfg():
    # small config for simulator validation; same code paths
    return dict(
        N=2000, E=8192, D=128, H=256, OUT=512, L=2, G=8, NCORES=8,
        RN=250, BLK=256, NSEG=4, CALL_TILES=4,
    )


def derive(cfg):
    cfg = dict(cfg)
    cfg["NW"] = cfg["BLK"] // P
    cfg["TROWS"] = cfg["NCORES"] * cfg["BLK"]
    assert cfg["TROWS"] % cfg["NSEG"] == 0
    cfg["SEGLEN"] = cfg["TROWS"] // cfg["NSEG"]
    assert cfg["SEGLEN"] <= 32767
    nw = cfg["NW"]
    base, extra = cfg["RN"] // nw, cfg["RN"] % nw
    cfg["SCHED"] = [base + 1 if w < extra else base for w in range(nw)]
    assert max(cfg["SCHED"]) <= P
    return cfg


# --------------------------------------------------------------------------
# host-side preprocessing (pure index/metadata manipulation)
# --------------------------------------------------------------------------

def assign_nodes(cfg, indeg):
    """Degree-balanced LPT: node -> (core, window-slot position).

    Bins are the (core, window) pairs; each bin holds exactly SCHED[w] real
    nodes (program-uniform). Returns (node2core, node2pos)."""
    import heapq
    c = cfg
    nw, ncores = c["NW"], c["NCORES"]
    sched = c["SCHED"]
    heap = []
    for core in range(ncores):
        for w in range(nw):
            heap.append((0.0, core * nw + w))
    heapq.heapify(heap)
    fill = np.zeros(ncores * nw, np.int64)
    n = len(indeg)
    node2core = np.empty(n, np.int64)
    node2pos = np.empty(n, np.int64)
    order = np.argsort(-indeg, kind="stable")
    for v in order:
        while True:
            load, b = heapq.heappop(heap)
            w = b % nw
            if fill[b] < sched[w]:
                break
        node2core[v] = b // nw
        node2pos[v] = w * P + fill[b]
        fill[b] += 1
        if fill[b] < sched[w]:
            heapq.heappush(heap, (load + float(indeg[v]), b))
    return node2core, node2pos


def preprocess(cfg, edge_index, batch):
    c = cfg
    src = np.asarray(edge_index[0], dtype=np.int64)
    dst = np.asarray(edge_index[1], dtype=np.int64)
    batch = np.asarray(batch, dtype=np.int64)

    indeg = np.bincount(dst, minlength=c["N"])
    node2core, node2pos = assign_nodes(c, indeg)

    owner = node2core[dst]
    src_pad = node2core[src] * c["BLK"] + node2pos[src]
    seg = src_pad // c["SEGLEN"]
    rel = (src_pad % c["SEGLEN"]).astype(np.int64)
    dl = node2pos[dst]                          # dst local slot position
    w = dl // P

    # per-core per-(seg, window) edge lists
    ncores, nseg, nw = c["NCORES"], c["NSEG"], c["NW"]
    counts = np.zeros((ncores, nseg, nw), np.int64)
    np.add.at(counts, (owner, seg, w), 1)
    ntiles = np.maximum.reduce(
        [(counts[k] + P - 1) // P for k in range(ncores)])  # [nseg, nw]

    runs = []            # ordered (s, w, nt, tile_offset)
    t_off = 0
    for s in range(nseg):
        for wi in range(nw):
            nt = int(ntiles[s, wi])
            if nt == 0:
                continue
            runs.append((s, wi, nt, t_off))
            t_off += nt
    ntt = t_off

    # calls: chunk run tiles within each segment
    calls = []           # (s, t0, t1)
    for s in range(nseg):
        seg_runs = [r for r in runs if r[0] == s]
        if not seg_runs:
            continue
        t0 = seg_runs[0][3]
        t_end = seg_runs[-1][3] + seg_runs[-1][2]
        t = t0
        while t < t_end:
            t1 = min(t + c["CALL_TILES"], t_end)
            calls.append((s, t, t1))
            t = t1

    # per-core data arrays
    order = np.lexsort((dl, seg, owner))        # owner-major, then seg, then dl
    o_s, s_s, r_s, d_s = owner[order], seg[order], rel[order], dl[order]

    gidx = np.zeros((ncores, P, ntt * 8), np.int16)
    s_u8 = np.zeros((ncores, ntt, P, P), ml_dtypes.bfloat16)

    run_index = {(s, wi): (nt, toff) for (s, wi, nt, toff) in runs}
    # boundaries of (core, seg, window) groups in the sorted edge list
    key = ((o_s * nseg + s_s) * nw + (d_s // P))
    bounds = np.flatnonzero(np.diff(key)) + 1
    starts = np.concatenate(([0], bounds))
    ends = np.concatenate((bounds, [len(key)]))
    for a, b in zip(starts, ends):
        core = int(o_s[a]); sg = int(s_s[a]); wi = int(d_s[a]) // P
        nt, toff = run_index[(sg, wi)]
        rels = r_s[a:b]
        dls = d_s[a:b] % P
        n = b - a
        assert n <= nt * P
        pos = np.arange(n)
        tt = toff + pos // P
        jj = pos % P
        s_u8[core, tt, jj, dls] = 1
        # wrapped idx layout: slot j of tile t -> row j%16, col t*8 + j//16
        cols = tt * 8 + jj // 16
        rows = jj % 16
        for repl in range(8):
            gidx[core, rows + 16 * repl, cols] = rels.astype(np.int16)
    return runs, calls, ntt, gidx, s_u8, node2core, node2pos


def build_host_inputs(cfg, inputs):
    """Returns (runs, calls, ntt, per_core_in_maps)."""
    c = cfg
    x = np.asarray(inputs["x"], np.float32)
    batch = np.asarray(inputs["batch"], np.int64)
    runs, calls, ntt, gidx, s_u8, node2core, node2pos = preprocess(
        c, inputs["edge_index"], batch)

    L, H, D, OUT, G = c["L"], c["H"], c["D"], c["OUT"], c["G"]
    node_w = np.asarray(inputs["node_w"], np.float32)
    node_b = np.asarray(inputs["node_b"], np.float32)
    gw1 = np.asarray(inputs["gin_w1"], np.float32)
    gb1 = np.asarray(inputs["gin_b1"], np.float32)
    gw2 = np.asarray(inputs["gin_w2"], np.float32)
    gb2 = np.asarray(inputs["gin_b2"], np.float32)
    eps = np.asarray(inputs["eps"], np.float32)
    ow1 = np.asarray(inputs["out_w1"], np.float32)
    ob1 = np.asarray(inputs["out_b1"], np.float32)
    ow2 = np.asarray(inputs["out_w2"], np.float32)
    ob2 = np.asarray(inputs["out_b2"], np.float32)

    cnt = np.bincount(batch, minlength=G).astype(np.float32)

    common = {
        "wpT": np.ascontiguousarray(node_w.T),              # [D, H]
        "bpT": np.ascontiguousarray(node_b.reshape(H // P, P).T),   # [P, H/P]
        "w1T": np.ascontiguousarray(np.transpose(gw1, (0, 2, 1))),  # [L, H, H]
        "b1T": np.ascontiguousarray(np.transpose(
            gb1.reshape(L, H // P, P), (0, 2, 1))),          # [L, P, H/P]
        "w2T": np.ascontiguousarray(np.transpose(gw2, (0, 2, 1))),
        "b2T": np.ascontiguousarray(np.transpose(
            gb2.reshape(L, H // P, P), (0, 2, 1))),
        "eps_rep": np.tile(eps.reshape(1, L), (P, 1)).astype(np.float32),
        "wo1T": np.ascontiguousarray(ow1.T),                # [H, H]
        "bo1T": np.ascontiguousarray(ob1.reshape(H // P, P).T),
        "wo2T": np.ascontiguousarray(ow2.T),                # [H, OUT]
        "bo2T": np.ascontiguousarray(ob2.reshape(OUT // P, P).T),   # [P, OUT/P]
        "cnt_rep": np.tile(cnt.reshape(1, G), (P, 1)),
    }

    in_maps = []
    for core in range(c["NCORES"]):
        mine = np.flatnonzero(node2core == core)
        pos = node2pos[mine]
        xo = np.zeros((D, c["BLK"]), np.float32)
        xo[:, pos] = x[mine].T
        gT = np.zeros((c["BLK"], G), ml_dtypes.bfloat16)
        gT[pos, batch[mine]] = 1.0
        m = dict(common)
        m["x_own"] = xo
        m["gidx"] = gidx[core]
        m["s_u8"] = s_u8[core]
        m["gT"] = gT
        in_maps.append(m)
    return runs, calls, ntt, in_maps


# --------------------------------------------------------------------------
# device program
# --------------------------------------------------------------------------

def build_program(cfg, runs, calls, ntt):
    c = cfg
    L, H, D, OUT, G = c["L"], c["H"], c["D"], c["OUT"], c["G"]
    NW, BLK, RN = c["NW"], c["BLK"], c["RN"]
    NH = H // P       # 2 channel halves
    NO = OUT // P     # 4 output quarters
    f32 = mybir.dt.float32
    bf16 = mybir.dt.bfloat16

    nc = bacc.Bacc("TRN2", target_bir_lowering=False, debug=False)

    x_own = nc.dram_tensor("x_own", [D, BLK], f32, kind="ExternalInput")
    gidx_d = nc.dram_tensor("gidx", [P, ntt * 8], mybir.dt.int16,
                            kind="ExternalInput")
    s_d = nc.dram_tensor("s_u8", [ntt, P, P], mybir.dt.bfloat16,
                         kind="ExternalInput")
    gT_d = nc.dram_tensor("gT", [BLK, G], bf16, kind="ExternalInput")
    wpT_d = nc.dram_tensor("wpT", [D, H], f32, kind="ExternalInput")
    bpT_d = nc.dram_tensor("bpT", [P, NH], f32, kind="ExternalInput")
    w1T_d = nc.dram_tensor("w1T", [L, H, H], f32, kind="ExternalInput")
    b1T_d = nc.dram_tensor("b1T", [L, P, NH], f32, kind="ExternalInput")
    w2T_d = nc.dram_tensor("w2T", [L, H, H], f32, kind="ExternalInput")
    b2T_d = nc.dram_tensor("b2T", [L, P, NH], f32, kind="ExternalInput")
    eps_d = nc.dram_tensor("eps_rep", [P, L], f32, kind="ExternalInput")
    wo1T_d = nc.dram_tensor("wo1T", [H, H], f32, kind="ExternalInput")
    bo1T_d = nc.dram_tensor("bo1T", [P, NH], f32, kind="ExternalInput")
    wo2T_d = nc.dram_tensor("wo2T", [H, OUT], f32, kind="ExternalInput")
    bo2T_d = nc.dram_tensor("bo2T", [P, NO], f32, kind="ExternalInput")
    cnt_d = nc.dram_tensor("cnt_rep", [P, G], f32, kind="ExternalInput")

    out_d = nc.dram_tensor("out", [G, OUT], f32, kind="ExternalOutput")

    h_own = nc.dram_tensor("h_own", [BLK, H], bf16)
    T_ag = nc.dram_tensor("T_ag", [c["TROWS"], H], bf16, addr_space="Shared")
    pp_in = nc.dram_tensor("pp_in", [P, NH * G], f32)
    pp_out = nc.dram_tensor("pp_out", [P, NH * G], f32, addr_space="Shared")

    rg = [list(range(c["NCORES"]))]

    with tile.TileContext(nc) as tc:
        with (
            tc.tile_pool(name="const", bufs=1) as cpool,
            tc.tile_pool(name="agg", bufs=1) as apool,
            tc.tile_pool(name="wt", bufs=2) as wpool,
            tc.tile_pool(name="sb", bufs=3) as sb,
            tc.tile_pool(name="gb", bufs=2) as gbp,
            tc.tile_pool(name="ps", bufs=4, space="PSUM") as ps,
            tc.tile_pool(name="ps_agg", bufs=2, space="PSUM") as ps_agg,
            tc.tile_pool(name="pool_ps", bufs=1, space="PSUM") as ppool,
        ):
            ident = cpool.tile([P, P], f32)
            make_identity(nc, ident[:])
            identb = cpool.tile([P, P], bf16)
            nc.vector.tensor_copy(identb[:], ident[:])
            eps_t = cpool.tile([P, L], f32)
            nc.sync.dma_start(out=eps_t[:], in_=eps_d[:])
            eps1p = cpool.tile([P, L], f32)
            nc.scalar.add(eps1p[:], eps_t[:], 1.0)

            # zero all of h_own once (pad slots sit inside every window)
            ZC = min(8, NW)
            zt = cpool.tile([P, ZC * H], bf16)
            nc.gpsimd.memset(zt[:], 0)
            assert BLK % (ZC * P) == 0
            for zb in range(BLK // (ZC * P)):
                nc.sync.dma_start(
                    out=h_own[zb * ZC * P:(zb + 1) * ZC * P, :].rearrange(
                        "(a p) c -> p a c", p=P),
                    in_=zt[:].rearrange("p (a c) -> p a c", c=H))

            sched = c["SCHED"]

            def rows_of(w):
                return sched[w]

            def mlp_pair(w0, zTp, w1sb, w2sb, b1sb, b2sb):
                """zTp [P, 2H] kk-major ([kk][wl][node]) for windows w0, w0+1."""
                H2 = 2 * H
                y1ps = ps.tile([P, H2], f32, space="PSUM", tag="mlp")
                for mh in range(NH):
                    for kk in range(NH):
                        nc.tensor.matmul(
                            out=y1ps[:, mh * 2 * P:(mh + 1) * 2 * P],
                            lhsT=w1sb[kk][:, mh * P:(mh + 1) * P],
                            rhs=zTp[:, kk * 2 * P:(kk + 1) * 2 * P],
                            start=(kk == 0), stop=(kk == NH - 1))
                y1 = sb.tile([P, H2], f32, tag="y1")
                for mh in range(NH):
                    nc.scalar.activation(
                        y1[:, mh * 2 * P:(mh + 1) * 2 * P],
                        y1ps[:, mh * 2 * P:(mh + 1) * 2 * P],
                        mybir.ActivationFunctionType.Relu,
                        bias=b1sb[:, mh:mh + 1], scale=1.0)
                y2ps = ps.tile([P, H2], f32, space="PSUM", tag="mlp")
                for mh in range(NH):
                    for kk in range(NH):
                        nc.tensor.matmul(
                            out=y2ps[:, mh * 2 * P:(mh + 1) * 2 * P],
                            lhsT=w2sb[kk][:, mh * P:(mh + 1) * P],
                            rhs=y1[:, kk * 2 * P:(kk + 1) * 2 * P],
                            start=(kk == 0), stop=(kk == NH - 1))
                h2 = sb.tile([P, H2], bf16, tag="h2")
                for mh in range(NH):
                    nc.scalar.activation(
                        h2[:, mh * 2 * P:(mh + 1) * 2 * P],
                        y2ps[:, mh * 2 * P:(mh + 1) * 2 * P],
                        mybir.ActivationFunctionType.Relu,
                        bias=b2sb[:, mh:mh + 1], scale=1.0)
                for wl in range(2):
                    w = w0 + wl
                    hnm = sb.tile([P, H], bf16, tag="hnm")
                    for mh in range(NH):
                        nc.sync.dma_start_transpose(
                            out=hnm[:, mh * P:(mh + 1) * P],
                            in_=h2[:, mh * 2 * P + wl * P:
                                   mh * 2 * P + (wl + 1) * P])
                    r = rows_of(w)
                    nc.sync.dma_start(out=h_own[w * P:w * P + r, :],
                                      in_=hnm[:r, :])

            def write_node_major(w, hfm):
                """hfm [P, H] feature-major -> transpose -> h_own window w."""
                htps = ps.tile([P, H], f32, space="PSUM", tag="mlp")
                for mh in range(NH):
                    nc.tensor.matmul(
                        out=htps[:, mh * P:(mh + 1) * P],
                        lhsT=hfm[:, mh * P:(mh + 1) * P], rhs=ident[:],
                        is_transpose=True, start=True, stop=True)
                hnm = sb.tile([P, H], bf16, tag="hnm")
                nc.vector.tensor_copy(hnm[:], htps[:])
                r = rows_of(w)
                nc.sync.dma_start(out=h_own[w * P:w * P + r, :], in_=hnm[:r, :])

            # ---------------- projection ----------------
            wp_sb = wpool.tile([D, H], f32, tag="wp")
            nc.sync.dma_start(out=wp_sb[:], in_=wpT_d[:])
            bp_sb = wpool.tile([P, NH], f32, tag="bp")
            nc.sync.dma_start(out=bp_sb[:], in_=bpT_d[:])
            CW = min(4, NW)  # windows per projection chunk
            for wc in range(0, NW, CW):
                cw = min(CW, NW - wc)
                xch = sb.tile([P, CW * P], f32, tag="xch")
                nc.sync.dma_start(
                    out=xch[:, :cw * P],
                    in_=x_own[:, wc * P:(wc + cw) * P])
                hps = []
                for mh in range(NH):
                    hps_t = ps.tile([P, CW * P], f32, space="PSUM",
                                    tag="mlp", name=f"hps{mh}")
                    nc.tensor.matmul(out=hps_t[:, :cw * P],
                                     lhsT=wp_sb[:, mh * P:(mh + 1) * P],
                                     rhs=xch[:, :cw * P], start=True, stop=True)
                    hps.append(hps_t)
                h0 = []
                for mh in range(NH):
                    h0_t = sb.tile([P, CW * P], bf16, tag="h0", name=f"h0{mh}")
                    nc.scalar.activation(
                        h0_t[:, :cw * P], hps[mh][:, :cw * P],
                        mybir.ActivationFunctionType.Relu,
                        bias=bp_sb[:, mh:mh + 1], scale=1.0)
                    h0.append(h0_t)
                for wl in range(cw):
                    w = wc + wl
                    hnm = sb.tile([P, H], bf16, tag="hnm")
                    for mh in range(NH):
                        nc.sync.dma_start_transpose(
                            out=hnm[:, mh * P:(mh + 1) * P],
                            in_=h0[mh][:, wl * P:(wl + 1) * P])
                    r = rows_of(w)
                    nc.sync.dma_start(out=h_own[w * P:w * P + r, :],
                                      in_=hnm[:r, :])

            # ---------------- GIN layers ----------------
            first_seg = {}
            for (s, wi, nt, toff) in runs:
                first_seg.setdefault(wi, s)

            for l in range(L):
                nc.gpsimd.collective_compute(
                    "AllGather", mybir.AluOpType.bypass,
                    replica_groups=rg, ins=[h_own[:]], outs=[T_ag[:]])

                agg = apool.tile([P, NW * H], f32, tag="agg")

                run_by_t = {}
                for (s, wi, nt, toff) in runs:
                    for t in range(toff, toff + nt):
                        run_by_t[t] = (s, wi, nt, toff)

                for (s, t0, t1) in calls:
                    ct = t1 - t0
                    gb = gbp.tile([P, ct * H], bf16, tag="gbuf")
                    idxt = sb.tile([P, ct * 8], mybir.dt.int16, tag="idxt")
                    nc.sync.dma_start(out=idxt[:],
                                      in_=gidx_d[:, t0 * 8:t1 * 8])
                    nc.gpsimd.dma_gather(
                        out_ap=gb[:].rearrange("p (t d) -> p t d", d=H),
                        in_ap=T_ag[s * c["SEGLEN"]:(s + 1) * c["SEGLEN"], :],
                        idxs_ap=idxt[:],
                        num_idxs=ct * P, num_idxs_reg=ct * P, elem_size=H)
                    ssb = gbp.tile([P, ct * P], bf16, tag="stile")
                    nc.sync.dma_start(
                        out=ssb[:].rearrange("e (t d) -> e t d", d=P),
                        in_=s_d[t0:t1].rearrange("t e d -> e t d"))
                    for t in range(t0, t1):
                        s_, wi, nt, toff = run_by_t[t]
                        if t == toff:
                            run_ps = ps_agg.tile([P, H], f32, space="PSUM",
                                                 tag="aggps")
                            run_by_t[(s_, wi, "ps")] = run_ps
                        run_ps = run_by_t[(s_, wi, "ps")]
                        nc.tensor.matmul(
                            out=run_ps[:],
                            lhsT=ssb[:, (t - t0) * P:(t - t0 + 1) * P],
                            rhs=gb[:, (t - t0) * H:(t - t0 + 1) * H],
                            start=(t == toff), stop=(t == toff + nt - 1))
                        if t == toff + nt - 1:
                            wsl = agg[:, wi * H:(wi + 1) * H]
                            if first_seg[wi] == s_:
                                nc.vector.tensor_copy(wsl, run_ps[:])
                            else:
                                nc.vector.tensor_add(wsl, wsl, run_ps[:])

                # weights for this layer
                w1sb = []
                w2sb = []
                for kk in range(NH):
                    t1w = wpool.tile([P, H], f32, tag=f"w1_{kk}")
                    nc.sync.dma_start(out=t1w[:],
                                      in_=w1T_d[l, kk * P:(kk + 1) * P, :])
                    w1sb.append(t1w)
                    t2w = wpool.tile([P, H], f32, tag=f"w2_{kk}")
                    nc.sync.dma_start(out=t2w[:],
                                      in_=w2T_d[l, kk * P:(kk + 1) * P, :])
                    w2sb.append(t2w)
                b1sb = wpool.tile([P, NH], f32, tag="b1")
                nc.sync.dma_start(out=b1sb[:], in_=b1T_d[l])
                b2sb = wpool.tile([P, NH], f32, tag="b2")
                nc.sync.dma_start(out=b2sb[:], in_=b2T_d[l])
                ieps = wpool.tile([P, P], bf16, tag="ieps")
                nc.scalar.activation(ieps[:], identb[:],
                                     mybir.ActivationFunctionType.Copy,
                                     bias=0.0, scale=eps1p[:, l:l + 1])

                assert NW % 2 == 0
                for w0 in range(0, NW, 2):
                    zTp = sb.tile([P, 2 * H], f32, tag="zTp")
                    for wl in range(2):
                        w = w0 + wl
                        hw = sb.tile([P, H], bf16, tag="hw")
                        nc.sync.dma_start(out=hw[:],
                                          in_=h_own[w * P:(w + 1) * P, :])
                        zps = ps.tile([P, H], f32, space="PSUM", tag="mlp")
                        for kk in range(NH):
                            nc.tensor.matmul(
                                out=zps[:, kk * P:(kk + 1) * P],
                                lhsT=agg[:, w * H + kk * P:
                                         w * H + (kk + 1) * P],
                                rhs=ident[:], is_transpose=True,
                                start=True, stop=False)
                            nc.tensor.matmul(
                                out=zps[:, kk * P:(kk + 1) * P],
                                lhsT=hw[:, kk * P:(kk + 1) * P], rhs=ieps[:],
                                start=False, stop=True)
                        for kk in range(NH):
                            nc.vector.tensor_copy(
                                zTp[:, kk * 2 * P + wl * P:
                                       kk * 2 * P + (wl + 1) * P],
                                zps[:, kk * P:(kk + 1) * P])
                    mlp_pair(w0, zTp, w1sb, w2sb, b1sb, b2sb)

            # ---------------- pooling + head ----------------
            pps = []
            for mh in range(NH):
                pps_t = ppool.tile([P, G], f32, space="PSUM",
                                   tag=f"pps{mh}", name=f"pps{mh}")
                pps.append(pps_t)
            for w in range(NW):
                hw = sb.tile([P, H], bf16, tag="hw2")
                nc.sync.dma_start(out=hw[:], in_=h_own[w * P:(w + 1) * P, :])
                gtw = sb.tile([P, G], bf16, tag="gtw")
                nc.sync.dma_start(out=gtw[:], in_=gT_d[w * P:(w + 1) * P, :])
                for mh in range(NH):
                    nc.tensor.matmul(
                        out=pps[mh][:],
                        lhsT=hw[:, mh * P:(mh + 1) * P], rhs=gtw[:],
                        start=(w == 0), stop=(w == NW - 1))
            psb = sb.tile([P, NH * G], f32, tag="psb")
            for mh in range(NH):
                nc.vector.tensor_copy(psb[:, mh * G:(mh + 1) * G], pps[mh][:])
            nc.sync.dma_start(out=pp_in[:], in_=psb[:])
            nc.gpsimd.collective_compute(
                "AllReduce", mybir.AluOpType.add,
                replica_groups=rg, ins=[pp_in[:]], outs=[pp_out[:]])
            ppsb = sb.tile([P, NH * G], f32, tag="ppsb")
            nc.sync.dma_start(out=ppsb[:], in_=pp_out[:])

            cntsb = cpool.tile([P, G], f32)
            nc.sync.dma_start(out=cntsb[:], in_=cnt_d[:])
            cnt2 = cpool.tile([P, G], f32)
            nc.vector.tensor_scalar(out=cnt2[:], in0=cntsb[:], scalar1=1.0,
                                    scalar2=None, op0=mybir.AluOpType.max)
            rec = cpool.tile([P, G], f32)
            nc.vector.reciprocal(rec[:], cnt2[:])
            hg = sb.tile([P, NH * G], f32, tag="hg")
            for mh in range(NH):
                nc.vector.tensor_mul(hg[:, mh * G:(mh + 1) * G],
                                      ppsb[:, mh * G:(mh + 1) * G], rec[:])

            wo1sb = []
            wo2sb = []
            for kk in range(NH):
                t1w = wpool.tile([P, H], f32, tag=f"wo1_{kk}")
                nc.sync.dma_start(out=t1w[:],
                                  in_=wo1T_d[kk * P:(kk + 1) * P, :])
                wo1sb.append(t1w)
                t2w = wpool.tile([P, OUT], f32, tag=f"wo2_{kk}")
                nc.sync.dma_start(out=t2w[:],
                                  in_=wo2T_d[kk * P:(kk + 1) * P, :])
                wo2sb.append(t2w)
            bo1sb = wpool.tile([P, NH], f32, tag="bo1")
            nc.sync.dma_start(out=bo1sb[:], in_=bo1T_d[:])
            bo2sb = wpool.tile([P, NO], f32, tag="bo2")
            nc.sync.dma_start(out=bo2sb[:], in_=bo2T_d[:])

            o1ps = ps.tile([P, NH * G], f32, space="PSUM", tag="mlp")
            for mh in range(NH):
                for kk in range(NH):
                    nc.tensor.matmul(
                        out=o1ps[:, mh * G:(mh + 1) * G],
                        lhsT=wo1sb[kk][:, mh * P:(mh + 1) * P],
                        rhs=hg[:, kk * G:(kk + 1) * G],
                        start=(kk == 0), stop=(kk == NH - 1))
            o1 = sb.tile([P, NH * G], f32, tag="o1")
            for mh in range(NH):
                nc.scalar.activation(
                    o1[:, mh * G:(mh + 1) * G], o1ps[:, mh * G:(mh + 1) * G],
                    mybir.ActivationFunctionType.Relu,
                    bias=bo1sb[:, mh:mh + 1], scale=1.0)
            o2ps = ps.tile([P, NO * G], f32, space="PSUM", tag="mlp")
            for mq in range(NO):
                for kk in range(NH):
                    nc.tensor.matmul(
                        out=o2ps[:, mq * G:(mq + 1) * G],
                        lhsT=wo2sb[kk][:, mq * P:(mq + 1) * P],
                        rhs=o1[:, kk * G:(kk + 1) * G],
                        start=(kk == 0), stop=(kk == NH - 1))
            o2 = sb.tile([P, NO * G], f32, tag="o2")
            for mq in range(NO):
                nc.vector.tensor_scalar_add(
                    o2[:, mq * G:(mq + 1) * G], o2ps[:, mq * G:(mq + 1) * G],
                    bo2sb[:, mq:mq + 1])
            otps = ps.tile([G, OUT], f32, space="PSUM", tag="mlp")
            for mq in range(NO):
                nc.tensor.matmul(
                    out=otps[:, mq * P:(mq + 1) * P],
                    lhsT=o2[:, mq * G:(mq + 1) * G], rhs=ident[:],
                    is_transpose=True, start=True, stop=True)
            osb = sb.tile([G, OUT], f32, tag="osb")
            nc.vector.tensor_copy(osb[:], otps[:])
            nc.sync.dma_start(out=out_d[:], in_=osb[:])

    nc.compile()
    return nc


# --------------------------------------------------------------------------
# public entry
# --------------------------------------------------------------------------

def run(cfg, inputs, mode="hw", trace=False):
    cfg = derive(cfg)
    runs, calls, ntt, in_maps = build_host_inputs(cfg, inputs)
    nc = build_program(cfg, runs, calls, ntt)
    if mode == "sim":
        from concourse.bass_interp import MultiCoreSim
        sim = MultiCoreSim(nc, num_cores=cfg["NCORES"])
        for cid, core in sim.cores.items():
            for k, v in in_maps[cid].items():
                core.tensor(k)[:] = v
        sim.simulate()
        return np.array(sim.cores[0].mem_tensor("out")), None
    from concourse.bass_utils import run_bass_kernel_spmd
    res = run_bass_kernel_spmd(nc, in_maps, list(range(cfg["NCORES"])),
                               trace=trace)
    return np.asarray(res.results[0]["out"]), res


def kernel(**inputs):
    out, _ = run(full_cfg(), inputs, mode="hw", trace=False)
    return out


# revision 28
# speedup vs baseline: 1.2729x; 1.2729x over previous
"""GIN graph encoder (DispatchGraphEncoder) on 8 Trainium2 NeuronCores.

Strategy (node-sharded, SPMD, all fp32):
- Nodes split contiguously across 8 cores (12500 each, padded to 12544).
- Per layer: ncfw AllGather replicates h into a full table T on every core.
- Aggregation: each core's incident edges are sorted by (src-segment, dst),
  padded into 128-edge tiles that are src-segment-pure and dst-window-pure.
  dma_gather (int16 idx relative to a <=32767-row table segment) pulls the
  1KB source-feature rows; a host-built one-hot selection matrix S
  (shipped uint8, cast to f32 during DMA) right-multiplies each tile on the
  PE, accumulating per-dst-window sums in PSUM; windows accumulate across
  segments in an SBUF agg buffer.
- z = agg + (1+eps)h is built transposed (feature-major) in PSUM via
  PE transpose-matmuls; the 2-layer GIN MLP runs feature-major (weights as
  lhsT), relu+bias fused on the ACT engine; output is transposed back and
  written node-major for the next AllGather.
- Pooling: matmul against a host-built graph-membership matrix, tiny
  AllReduce, replicated final head; core 0's [64, 512] output is returned.

The SPMD program structure (tile counts per (segment, window) run) is made
uniform across cores by padding to the max core's count; dead slots carry
idx=0 and an all-zero S column.
"""
import os
import sys

import numpy as np
import ml_dtypes

sys.path.insert(0, "/opt/trn_rl_repo")

from concourse import bass, bacc, mybir, tile  # noqa: E402
from concourse.masks import make_identity  # noqa: E402

P = 128


def full_cfg():
    return dict(
        N=100000, E=800000, D=128, H=256, OUT=512, L=4, G=64, NCORES=8,
        RN=12500, BLK=14336, NSEG=4, CALL_TILES=8,
    )


def tiny_cfg():
    # small config for simulator validation; same code paths
    return dict(
        N=2000, E=8192, D=128, H=256, OUT=512, L=2, G=8, NCORES=8,
        RN=250, BLK=256, NSEG=4, CALL_TILES=4,
    )


def derive(cfg):
    cfg = dict(cfg)
    cfg["NW"] = cfg["BLK"] // P
    cfg["TROWS"] = cfg["NCORES"] * cfg["BLK"]
    assert cfg["TROWS"] % cfg["NSEG"] == 0
    cfg["SEGLEN"] = cfg["TROWS"] // cfg["NSEG"]
    assert cfg["SEGLEN"] <= 32767
    nw = cfg["NW"]
    base, extra = cfg["RN"] // nw, cfg["RN"] % nw
    cfg["SCHED"] = [base + 1 if w < extra else base for w in range(nw)]
    assert max(cfg["SCHED"]) <= P
    return cfg


# --------------------------------------------------------------------------
# host-side preprocessing (pure index/metadata manipulation)
# --------------------------------------------------------------------------

def assign_nodes(cfg, indeg):
    """Degree-balanced LPT: node -> (core, window-slot position).

    Bins are the (core, window) pairs; each bin holds exactly SCHED[w] real
    nodes (program-uniform). Returns (node2core, node2pos)."""
    import heapq
    c = cfg
    nw, ncores = c["NW"], c["NCORES"]
    sched = c["SCHED"]
    heap = []
    for core in range(ncores):
        for w in range(nw):
            heap.append((0.0, core * nw + w))
    heapq.heapify(heap)
    fill = np.zeros(ncores * nw, np.int64)
    n = len(indeg)
    node2core = np.empty(n, np.int64)
    node2pos = np.empty(n, np.int64)
    order = np.argsort(-indeg, kind="stable")
    for v in order:
        while True:
            load, b = heapq.heappop(heap)
            w = b % nw
            if fill[b] < sched[w]:
                break
        node2core[v] = b // nw
        node2pos[v] = w * P + fill[b]
        fill[b] += 1
        if fill[b] < sched[w]:
            heapq.heappush(heap, (load + float(indeg[v]), b))
    return node2core, node2pos


def preprocess(cfg, edge_index, batch):
    c = cfg
    src = np.asarray(edge_index[0], dtype=np.int64)
    dst = np.asarray(edge_index[1], dtype=np.int64)
    batch = np.asarray(batch, dtype=np.int64)

    indeg = np.bincount(dst, minlength=c["N"])
    node2core, node2pos = assign_nodes(c, indeg)

    owner = node2core[dst]
    src_pad = node2core[src] * c["BLK"] + node2pos[src]
    seg = src_pad // c["SEGLEN"]
    rel = (src_pad % c["SEGLEN"]).astype(np.int64)
    dl = node2pos[dst]                          # dst local slot position
    w = dl // P

    # per-core per-(seg, window) edge lists
    ncores, nseg, nw = c["NCORES"], c["NSEG"], c["NW"]
    counts = np.zeros((ncores, nseg, nw), np.int64)
    np.add.at(counts, (owner, seg, w), 1)
    ntiles = np.maximum.reduce(
        [(counts[k] + P - 1) // P for k in range(ncores)])  # [nseg, nw]

    runs = []            # ordered (s, w, nt, tile_offset)
    t_off = 0
    for s in range(nseg):
        for wi in range(nw):
            nt = int(ntiles[s, wi])
            if nt == 0:
                continue
            runs.append((s, wi, nt, t_off))
            t_off += nt
    ntt = t_off

    # calls: chunk run tiles within each segment
    calls = []           # (s, t0, t1)
    for s in range(nseg):
        seg_runs = [r for r in runs if r[0] == s]
        if not seg_runs:
            continue
        t0 = seg_runs[0][3]
        t_end = seg_runs[-1][3] + seg_runs[-1][2]
        t = t0
        while t < t_end:
            t1 = min(t + c["CALL_TILES"], t_end)
            calls.append((s, t, t1))
            t = t1

    # per-core data arrays
    order = np.lexsort((dl, seg, owner))        # owner-major, then seg, then dl
    o_s, s_s, r_s, d_s = owner[order], seg[order], rel[order], dl[order]

    gidx = np.zeros((ncores, P, ntt * 8), np.int16)
    s_u8 = np.zeros((ncores, ntt, P, P), ml_dtypes.bfloat16)

    run_index = {(s, wi): (nt, toff) for (s, wi, nt, toff) in runs}
    # boundaries of (core, seg, window) groups in the sorted edge list
    key = ((o_s * nseg + s_s) * nw + (d_s // P))
    bounds = np.flatnonzero(np.diff(key)) + 1
    starts = np.concatenate(([0], bounds))
    ends = np.concatenate((bounds, [len(key)]))
    for a, b in zip(starts, ends):
        core = int(o_s[a]); sg = int(s_s[a]); wi = int(d_s[a]) // P
        nt, toff = run_index[(sg, wi)]
        rels = r_s[a:b]
        dls = d_s[a:b] % P
        n = b - a
        assert n <= nt * P
        pos = np.arange(n)
        tt = toff + pos // P
        jj = pos % P
        s_u8[core, tt, jj, dls] = 1
        # wrapped idx layout: slot j of tile t -> row j%16, col t*8 + j//16
        cols = tt * 8 + jj // 16
        rows = jj % 16
        for repl in range(8):
            gidx[core, rows + 16 * repl, cols] = rels.astype(np.int16)
    return runs, calls, ntt, gidx, s_u8, node2core, node2pos


def build_host_inputs(cfg, inputs):
    """Returns (runs, calls, ntt, per_core_in_maps)."""
    c = cfg
    x = np.asarray(inputs["x"], np.float32)
    batch = np.asarray(inputs["batch"], np.int64)
    runs, calls, ntt, gidx, s_u8, node2core, node2pos = preprocess(
        c, inputs["edge_index"], batch)

    L, H, D, OUT, G = c["L"], c["H"], c["D"], c["OUT"], c["G"]
    node_w = np.asarray(inputs["node_w"], np.float32)
    node_b = np.asarray(inputs["node_b"], np.float32)
    gw1 = np.asarray(inputs["gin_w1"], np.float32)
    gb1 = np.asarray(inputs["gin_b1"], np.float32)
    gw2 = np.asarray(inputs["gin_w2"], np.float32)
    gb2 = np.asarray(inputs["gin_b2"], np.float32)
    eps = np.asarray(inputs["eps"], np.float32)
    ow1 = np.asarray(inputs["out_w1"], np.float32)
    ob1 = np.asarray(inputs["out_b1"], np.float32)
    ow2 = np.asarray(inputs["out_w2"], np.float32)
    ob2 = np.asarray(inputs["out_b2"], np.float32)

    cnt = np.bincount(batch, minlength=G).astype(np.float32)

    common = {
        "wpT": np.ascontiguousarray(node_w.T),              # [D, H]
        "bpT": np.ascontiguousarray(node_b.reshape(H // P, P).T),   # [P, H/P]
        "w1T": np.ascontiguousarray(np.transpose(gw1, (0, 2, 1))),  # [L, H, H]
        "b1T": np.ascontiguousarray(np.transpose(
            gb1.reshape(L, H // P, P), (0, 2, 1))),          # [L, P, H/P]
        "w2T": np.ascontiguousarray(np.transpose(gw2, (0, 2, 1))),
        "b2T": np.ascontiguousarray(np.transpose(
            gb2.reshape(L, H // P, P), (0, 2, 1))),
        "eps_rep": np.tile(eps.reshape(1, L), (P, 1)).astype(np.float32),
        "wo1T": np.ascontiguousarray(ow1.T),                # [H, H]
        "bo1T": np.ascontiguousarray(ob1.reshape(H // P, P).T),
        "wo2T": np.ascontiguousarray(ow2.T),                # [H, OUT]
        "bo2T": np.ascontiguousarray(ob2.reshape(OUT // P, P).T),   # [P, OUT/P]
        "cnt_rep": np.tile(cnt.reshape(1, G), (P, 1)),
    }

    in_maps = []
    for core in range(c["NCORES"]):
        mine = np.flatnonzero(node2core == core)
        pos = node2pos[mine]
        xo = np.zeros((D, c["BLK"]), np.float32)
        xo[:, pos] = x[mine].T
        gT = np.zeros((c["BLK"], G), ml_dtypes.bfloat16)
        gT[pos, batch[mine]] = 1.0
        m = dict(common)
        m["x_own"] = xo
        m["gidx"] = gidx[core]
        m["s_u8"] = s_u8[core]
        m["gT"] = gT
        in_maps.append(m)
    return runs, calls, ntt, in_maps


# --------------------------------------------------------------------------
# device program
# --------------------------------------------------------------------------

def build_program(cfg, runs, calls, ntt):
    c = cfg
    L, H, D, OUT, G = c["L"], c["H"], c["D"], c["OUT"], c["G"]
    NW, BLK, RN = c["NW"], c["BLK"], c["RN"]
    NH = H // P       # 2 channel halves
    NO = OUT // P     # 4 output quarters
    f32 = mybir.dt.float32
    bf16 = mybir.dt.bfloat16

    nc = bacc.Bacc("TRN2", target_bir_lowering=False, debug=False)

    x_own = nc.dram_tensor("x_own", [D, BLK], f32, kind="ExternalInput")
    gidx_d = nc.dram_tensor("gidx", [P, ntt * 8], mybir.dt.int16,
                            kind="ExternalInput")
    s_d = nc.dram_tensor("s_u8", [ntt, P, P], mybir.dt.bfloat16,
                         kind="ExternalInput")
    gT_d = nc.dram_tensor("gT", [BLK, G], bf16, kind="ExternalInput")
    wpT_d = nc.dram_tensor("wpT", [D, H], f32, kind="ExternalInput")
    bpT_d = nc.dram_tensor("bpT", [P, NH], f32, kind="ExternalInput")
    w1T_d = nc.dram_tensor("w1T", [L, H, H], f32, kind="ExternalInput")
    b1T_d = nc.dram_tensor("b1T", [L, P, NH], f32, kind="ExternalInput")
    w2T_d = nc.dram_tensor("w2T", [L, H, H], f32, kind="ExternalInput")
    b2T_d = nc.dram_tensor("b2T", [L, P, NH], f32, kind="ExternalInput")
    eps_d = nc.dram_tensor("eps_rep", [P, L], f32, kind="ExternalInput")
    wo1T_d = nc.dram_tensor("wo1T", [H, H], f32, kind="ExternalInput")
    bo1T_d = nc.dram_tensor("bo1T", [P, NH], f32, kind="ExternalInput")
    wo2T_d = nc.dram_tensor("wo2T", [H, OUT], f32, kind="ExternalInput")
    bo2T_d = nc.dram_tensor("bo2T", [P, NO], f32, kind="ExternalInput")
    cnt_d = nc.dram_tensor("cnt_rep", [P, G], f32, kind="ExternalInput")

    out_d = nc.dram_tensor("out", [G, OUT], f32, kind="ExternalOutput")

    h_own = nc.dram_tensor("h_own", [BLK, H], bf16)
    T_ag = nc.dram_tensor("T_ag", [c["TROWS"], H], bf16, addr_space="Shared")
    pp_in = nc.dram_tensor("pp_in", [P, NH * G], f32)
    pp_out = nc.dram_tensor("pp_out", [P, NH * G], f32, addr_space="Shared")

    rg = [list(range(c["NCORES"]))]

    with tile.TileContext(nc) as tc:
        with (
            tc.tile_pool(name="const", bufs=1) as cpool,
            tc.tile_pool(name="agg", bufs=1) as apool,
            tc.tile_pool(name="wt", bufs=2) as wpool,
            tc.tile_pool(name="sb", bufs=3) as sb,
            tc.tile_pool(name="gb", bufs=2) as gbp,
            tc.tile_pool(name="ps", bufs=4, space="PSUM") as ps,
            tc.tile_pool(name="ps_agg", bufs=2, space="PSUM") as ps_agg,
            tc.tile_pool(name="pool_ps", bufs=1, space="PSUM") as ppool,
        ):
            ident = cpool.tile([P, P], f32)
            make_identity(nc, ident[:])
            identb = cpool.tile([P, P], bf16)
            nc.vector.tensor_copy(identb[:], ident[:])
            eps_t = cpool.tile([P, L], f32)
            nc.sync.dma_start(out=eps_t[:], in_=eps_d[:])
            eps1p = cpool.tile([P, L], f32)
            nc.scalar.add(eps1p[:], eps_t[:], 1.0)

            # zero all of h_own once (pad slots sit inside every window)
            ZC = min(8, NW)
            zt = cpool.tile([P, ZC * H], bf16)
            nc.gpsimd.memset(zt[:], 0)
            assert BLK % (ZC * P) == 0
            for zb in range(BLK // (ZC * P)):
                nc.sync.dma_start(
                    out=h_own[zb * ZC * P:(zb + 1) * ZC * P, :].rearrange(
                        "(a p) c -> p a c", p=P),
                    in_=zt[:].rearrange("p (a c) -> p a c", c=H))

            sched = c["SCHED"]

            def rows_of(w):
                return sched[w]

            def mlp_pair(w0, zTp, w1sb, w2sb, b1sb, b2sb):
                """zTp [P, 2H] kk-major ([kk][wl][node]) for windows w0, w0+1."""
                H2 = 2 * H
                y1ps = ps.tile([P, H2], f32, space="PSUM", tag="mlp")
                for mh in range(NH):
                    for kk in range(NH):
                        nc.tensor.matmul(
                            out=y1ps[:, mh * 2 * P:(mh + 1) * 2 * P],
                            lhsT=w1sb[kk][:, mh * P:(mh + 1) * P],
                            rhs=zTp[:, kk * 2 * P:(kk + 1) * 2 * P],
                            start=(kk == 0), stop=(kk == NH - 1))
                y1 = sb.tile([P, H2], f32, tag="y1")
                for mh in range(NH):
                    nc.scalar.activation(
                        y1[:, mh * 2 * P:(mh + 1) * 2 * P],
                        y1ps[:, mh * 2 * P:(mh + 1) * 2 * P],
                        mybir.ActivationFunctionType.Relu,
                        bias=b1sb[:, mh:mh + 1], scale=1.0)
                y2ps = ps.tile([P, H2], f32, space="PSUM", tag="mlp")
                for mh in range(NH):
                    for kk in range(NH):
                        nc.tensor.matmul(
                            out=y2ps[:, mh * 2 * P:(mh + 1) * 2 * P],
                            lhsT=w2sb[kk][:, mh * P:(mh + 1) * P],
                            rhs=y1[:, kk * 2 * P:(kk + 1) * 2 * P],
                            start=(kk == 0), stop=(kk == NH - 1))
                h2 = sb.tile([P, H2], f32, tag="h2")
                for mh in range(NH):
                    nc.scalar.activation(
                        h2[:, mh * 2 * P:(mh + 1) * 2 * P],
                        y2ps[:, mh * 2 * P:(mh + 1) * 2 * P],
                        mybir.ActivationFunctionType.Relu,
                        bias=b2sb[:, mh:mh + 1], scale=1.0)
                for wl in range(2):
                    w = w0 + wl
                    htps = ps.tile([P, H], f32, space="PSUM", tag="mlp")
                    for mh in range(NH):
                        nc.tensor.matmul(
                            out=htps[:, mh * P:(mh + 1) * P],
                            lhsT=h2[:, mh * 2 * P + wl * P:
                                    mh * 2 * P + (wl + 1) * P],
                            rhs=ident[:],
                            is_transpose=True, start=True, stop=True)
                    hnm = sb.tile([P, H], bf16, tag="hnm")
                    nc.vector.tensor_copy(hnm[:], htps[:])
                    r = rows_of(w)
                    nc.sync.dma_start(out=h_own[w * P:w * P + r, :],
                                      in_=hnm[:r, :])

            def write_node_major(w, hfm):
                """hfm [P, H] feature-major -> transpose -> h_own window w."""
                htps = ps.tile([P, H], f32, space="PSUM", tag="mlp")
                for mh in range(NH):
                    nc.tensor.matmul(
                        out=htps[:, mh * P:(mh + 1) * P],
                        lhsT=hfm[:, mh * P:(mh + 1) * P], rhs=ident[:],
                        is_transpose=True, start=True, stop=True)
                hnm = sb.tile([P, H], bf16, tag="hnm")
                nc.vector.tensor_copy(hnm[:], htps[:])
                r = rows_of(w)
                nc.sync.dma_start(out=h_own[w * P:w * P + r, :], in_=hnm[:r, :])

            # ---------------- projection ----------------
            wp_sb = wpool.tile([D, H], f32, tag="wp")
            nc.sync.dma_start(out=wp_sb[:], in_=wpT_d[:])
            bp_sb = wpool.tile([P, NH], f32, tag="bp")
            nc.sync.dma_start(out=bp_sb[:], in_=bpT_d[:])
            CW = min(4, NW)  # windows per projection chunk
            for wc in range(0, NW, CW):
                cw = min(CW, NW - wc)
                xch = sb.tile([P, CW * P], f32, tag="xch")
                nc.sync.dma_start(
                    out=xch[:, :cw * P],
                    in_=x_own[:, wc * P:(wc + cw) * P])
                hps = []
                for mh in range(NH):
                    hps_t = ps.tile([P, CW * P], f32, space="PSUM",
                                    tag="mlp", name=f"hps{mh}")
                    nc.tensor.matmul(out=hps_t[:, :cw * P],
                                     lhsT=wp_sb[:, mh * P:(mh + 1) * P],
                                     rhs=xch[:, :cw * P], start=True, stop=True)
                    hps.append(hps_t)
                h0 = []
                for mh in range(NH):
                    h0_t = sb.tile([P, CW * P], f32, tag="h0", name=f"h0{mh}")
                    nc.scalar.activation(
                        h0_t[:, :cw * P], hps[mh][:, :cw * P],
                        mybir.ActivationFunctionType.Relu,
                        bias=bp_sb[:, mh:mh + 1], scale=1.0)
                    h0.append(h0_t)
                for wl in range(cw):
                    w = wc + wl
                    htps = ps.tile([P, H], f32, space="PSUM", tag="mlp")
                    for mh in range(NH):
                        nc.tensor.matmul(
                            out=htps[:, mh * P:(mh + 1) * P],
                            lhsT=h0[mh][:, wl * P:(wl + 1) * P], rhs=ident[:],
                            is_transpose=True, start=True, stop=True)
                    hnm = sb.tile([P, H], bf16, tag="hnm")
                    nc.vector.tensor_copy(hnm[:], htps[:])
                    r = rows_of(w)
                    nc.sync.dma_start(out=h_own[w * P:w * P + r, :],
                                      in_=hnm[:r, :])

            # ---------------- GIN layers ----------------
            first_seg = {}
            for (s, wi, nt, toff) in runs:
                first_seg.setdefault(wi, s)

            for l in range(L):
                nc.gpsimd.collective_compute(
                    "AllGather", mybir.AluOpType.bypass,
                    replica_groups=rg, ins=[h_own[:]], outs=[T_ag[:]])

                agg = apool.tile([P, NW * H], f32, tag="agg")

                run_by_t = {}
                for (s, wi, nt, toff) in runs:
                    for t in range(toff, toff + nt):
                        run_by_t[t] = (s, wi, nt, toff)

                for (s, t0, t1) in calls:
                    ct = t1 - t0
                    gb = gbp.tile([P, ct * H], bf16, tag="gbuf")
                    idxt = sb.tile([P, ct * 8], mybir.dt.int16, tag="idxt")
                    nc.sync.dma_start(out=idxt[:],
                                      in_=gidx_d[:, t0 * 8:t1 * 8])
                    nc.gpsimd.dma_gather(
                        out_ap=gb[:].rearrange("p (t d) -> p t d", d=H),
                        in_ap=T_ag[s * c["SEGLEN"]:(s + 1) * c["SEGLEN"], :],
                        idxs_ap=idxt[:],
                        num_idxs=ct * P, num_idxs_reg=ct * P, elem_size=H)
                    ssb = gbp.tile([P, ct * P], bf16, tag="stile")
                    nc.sync.dma_start(
                        out=ssb[:].rearrange("e (t d) -> e t d", d=P),
                        in_=s_d[t0:t1].rearrange("t e d -> e t d"))
                    for t in range(t0, t1):
                        s_, wi, nt, toff = run_by_t[t]
                        if t == toff:
                            run_ps = ps_agg.tile([P, H], f32, space="PSUM",
                                                 tag="aggps")
                            run_by_t[(s_, wi, "ps")] = run_ps
                        run_ps = run_by_t[(s_, wi, "ps")]
                        nc.tensor.matmul(
                            out=run_ps[:],
                            lhsT=ssb[:, (t - t0) * P:(t - t0 + 1) * P],
                            rhs=gb[:, (t - t0) * H:(t - t0 + 1) * H],
                            start=(t == toff), stop=(t == toff + nt - 1))
                        if t == toff + nt - 1:
                            wsl = agg[:, wi * H:(wi + 1) * H]
                            if first_seg[wi] == s_:
                                nc.vector.tensor_copy(wsl, run_ps[:])
                            else:
                                nc.vector.tensor_add(wsl, wsl, run_ps[:])

                # weights for this layer
                w1sb = []
                w2sb = []
                for kk in range(NH):
                    t1w = wpool.tile([P, H], f32, tag=f"w1_{kk}")
                    nc.sync.dma_start(out=t1w[:],
                                      in_=w1T_d[l, kk * P:(kk + 1) * P, :])
                    w1sb.append(t1w)
                    t2w = wpool.tile([P, H], f32, tag=f"w2_{kk}")
                    nc.sync.dma_start(out=t2w[:],
                                      in_=w2T_d[l, kk * P:(kk + 1) * P, :])
                    w2sb.append(t2w)
                b1sb = wpool.tile([P, NH], f32, tag="b1")
                nc.sync.dma_start(out=b1sb[:], in_=b1T_d[l])
                b2sb = wpool.tile([P, NH], f32, tag="b2")
                nc.sync.dma_start(out=b2sb[:], in_=b2T_d[l])
                ieps = wpool.tile([P, P], bf16, tag="ieps")
                nc.scalar.activation(ieps[:], identb[:],
                                     mybir.ActivationFunctionType.Copy,
                                     bias=0.0, scale=eps1p[:, l:l + 1])

                assert NW % 2 == 0
                for w0 in range(0, NW, 2):
                    zTp = sb.tile([P, 2 * H], f32, tag="zTp")
                    for wl in range(2):
                        w = w0 + wl
                        hw = sb.tile([P, H], bf16, tag="hw")
                        nc.sync.dma_start(out=hw[:],
                                          in_=h_own[w * P:(w + 1) * P, :])
                        zps = ps.tile([P, H], f32, space="PSUM", tag="mlp")
                        for kk in range(NH):
                            nc.tensor.matmul(
                                out=zps[:, kk * P:(kk + 1) * P],
                                lhsT=agg[:, w * H + kk * P:
                                         w * H + (kk + 1) * P],
                                rhs=ident[:], is_transpose=True,
                                start=True, stop=False)
                            nc.tensor.matmul(
                                out=zps[:, kk * P:(kk + 1) * P],
                                lhsT=hw[:, kk * P:(kk + 1) * P], rhs=ieps[:],
                                start=False, stop=True)
                        for kk in range(NH):
                            nc.vector.tensor_copy(
                                zTp[:, kk * 2 * P + wl * P:
                                       kk * 2 * P + (wl + 1) * P],
                                zps[:, kk * P:(kk + 1) * P])
                    mlp_pair(w0, zTp, w1sb, w2sb, b1sb, b2sb)

            # ---------------- pooling + head ----------------
            pps = []
            for mh in range(NH):
                pps_t = ppool.tile([P, G], f32, space="PSUM",
                                   tag=f"pps{mh}", name=f"pps{mh}")
                pps.append(pps_t)
            for w in range(NW):
                hw = sb.tile([P, H], bf16, tag="hw2")
                nc.sync.dma_start(out=hw[:], in_=h_own[w * P:(w + 1) * P, :])
                gtw = sb.tile([P, G], bf16, tag="gtw")
                nc.sync.dma_start(out=gtw[:], in_=gT_d[w * P:(w + 1) * P, :])
                for mh in range(NH):
                    nc.tensor.matmul(
                        out=pps[mh][:],
                        lhsT=hw[:, mh * P:(mh + 1) * P], rhs=gtw[:],
                        start=(w == 0), stop=(w == NW - 1))
            psb = sb.tile([P, NH * G], f32, tag="psb")
            for mh in range(NH):
                nc.vector.tensor_copy(psb[:, mh * G:(mh + 1) * G], pps[mh][:])
            nc.sync.dma_start(out=pp_in[:], in_=psb[:])
            nc.gpsimd.collective_compute(
                "AllReduce", mybir.AluOpType.add,
                replica_groups=rg, ins=[pp_in[:]], outs=[pp_out[:]])
            ppsb = sb.tile([P, NH * G], f32, tag="ppsb")
            nc.sync.dma_start(out=ppsb[:], in_=pp_out[:])

            cntsb = cpool.tile([P, G], f32)
            nc.sync.dma_start(out=cntsb[:], in_=cnt_d[:])
            cnt2 = cpool.tile([P, G], f32)
            nc.vector.tensor_scalar(out=cnt2[:], in0=cntsb[:], scalar1=1.0,
                                    scalar2=None, op0=mybir.AluOpType.max)
            rec = cpool.tile([P, G], f32)
            nc.vector.reciprocal(rec[:], cnt2[:])
            hg = sb.tile([P, NH * G], f32, tag="hg")
            for mh in range(NH):
                nc.vector.tensor_mul(hg[:, mh * G:(mh + 1) * G],
                                      ppsb[:, mh * G:(mh + 1) * G], rec[:])

            wo1sb = []
            wo2sb = []
            for kk in range(NH):
                t1w = wpool.tile([P, H], f32, tag=f"wo1_{kk}")
                nc.sync.dma_start(out=t1w[:],
                                  in_=wo1T_d[kk * P:(kk + 1) * P, :])
                wo1sb.append(t1w)
                t2w = wpool.tile([P, OUT], f32, tag=f"wo2_{kk}")
                nc.sync.dma_start(out=t2w[:],
                                  in_=wo2T_d[kk * P:(kk + 1) * P, :])
                wo2sb.append(t2w)
            bo1sb = wpool.tile([P, NH], f32, tag="bo1")
            nc.sync.dma_start(out=bo1sb[:], in_=bo1T_d[:])
            bo2sb = wpool.tile([P, NO], f32, tag="bo2")
            nc.sync.dma_start(out=bo2sb[:], in_=bo2T_d[:])

            o1ps = ps.tile([P, NH * G], f32, space="PSUM", tag="mlp")
            for mh in range(NH):
                for kk in range(NH):
                    nc.tensor.matmul(
                        out=o1ps[:, mh * G:(mh + 1) * G],
                        lhsT=wo1sb[kk][:, mh * P:(mh + 1) * P],
                        rhs=hg[:, kk * G:(kk + 1) * G],
                        start=(kk == 0), stop=(kk == NH - 1))
            o1 = sb.tile([P, NH * G], f32, tag="o1")
            for mh in range(NH):
                nc.scalar.activation(
                    o1[:, mh * G:(mh + 1) * G], o1ps[:, mh * G:(mh + 1) * G],
                    mybir.ActivationFunctionType.Relu,
                    bias=bo1sb[:, mh:mh + 1], scale=1.0)
            o2ps = ps.tile([P, NO * G], f32, space="PSUM", tag="mlp")
            for mq in range(NO):
                for kk in range(NH):
                    nc.tensor.matmul(
                        out=o2ps[:, mq * G:(mq + 1) * G],
                        lhsT=wo2sb[kk][:, mq * P:(mq + 1) * P],
                        rhs=o1[:, kk * G:(kk + 1) * G],
                        start=(kk == 0), stop=(kk == NH - 1))
            o2 = sb.tile([P, NO * G], f32, tag="o2")
            for mq in range(NO):
                nc.vector.tensor_scalar_add(
                    o2[:, mq * G:(mq + 1) * G], o2ps[:, mq * G:(mq + 1) * G],
                    bo2sb[:, mq:mq + 1])
            otps = ps.tile([G, OUT], f32, space="PSUM", tag="mlp")
            for mq in range(NO):
                nc.tensor.matmul(
                    out=otps[:, mq * P:(mq + 1) * P],
                    lhsT=o2[:, mq * G:(mq + 1) * G], rhs=ident[:],
                    is_transpose=True, start=True, stop=True)
            osb = sb.tile([G, OUT], f32, tag="osb")
            nc.vector.tensor_copy(osb[:], otps[:])
            nc.sync.dma_start(out=out_d[:], in_=osb[:])

    nc.compile()
    return nc


# --------------------------------------------------------------------------
# public entry
# --------------------------------------------------------------------------

def run(cfg, inputs, mode="hw", trace=False):
    cfg = derive(cfg)
    runs, calls, ntt, in_maps = build_host_inputs(cfg, inputs)
    nc = build_program(cfg, runs, calls, ntt)
    if mode == "sim":
        from concourse.bass_interp import MultiCoreSim
        sim = MultiCoreSim(nc, num_cores=cfg["NCORES"])
        for cid, core in sim.cores.items():
            for k, v in in_maps[cid].items():
                core.tensor(k)[:] = v
        sim.simulate()
        return np.array(sim.cores[0].mem_tensor("out")), None
    from concourse.bass_utils import run_bass_kernel_spmd
    res = run_bass_kernel_spmd(nc, in_maps, list(range(cfg["NCORES"])),
                               trace=trace)
    return np.asarray(res.results[0]["out"]), res


def kernel(**inputs):
    out, _ = run(full_cfg(), inputs, mode="hw", trace=False)
    return out
